# revision 3
# baseline (speedup 1.0000x reference)
"""CrossNonLocal2D kernel for Trainium2, 8-way batch-parallel SPMD.

Per core (one batch element b), all matmuls in bf16 (fp32 PSUM accum):
  theta = theta_w @ xt + tb      [I, N]
  phi   = phi_w   @ xo + pb      [I, N]
  gT    = (g_w @ xo)^T           [N, I]   (computed directly transposed)
  ST    = phi^T @ theta          [m, n] tiles  (attention logits, transposed)
  PT    = exp(ST)                (no max subtraction -- logits bounded ~+-55)
  yu    = P @ [gT | 1]           [n, I+1]  (ones column gives softmax row-sum)
  y     = yu[:, :I] / yu[:, I]   then PE-transpose -> [I, n]
  out   = x_this + w_eff @ y + b_eff   (BN + g/out biases folded on host)

End-to-end numeric error vs fp32 reference: ~2.5e-4 (rel fro).
"""

import os
import sys
import time

import numpy as np

for _p in ("/opt/trn_rl_repo",):
    if os.path.isdir(_p) and _p not in sys.path:
        sys.path.insert(0, _p)

import ml_dtypes  # noqa: E402
import concourse.bacc as bacc  # noqa: E402
import concourse.mybir as mybir  # noqa: E402
import concourse.tile as tile  # noqa: E402
from concourse.bass import ts  # noqa: E402
from concourse.bass_utils import run_bass_kernel_spmd  # noqa: E402

B, C, HH, WW = 8, 256, 64, 64
N = HH * WW  # 4096
I = 128  # inter channels
NCORES = 8
BN_EPS = 1e-5
NCH = N // 512  # 8 n-chunks of 512
MT = N // 128  # 32 m-tiles of 128

f32 = mybir.dt.float32
bf16 = mybir.dt.bfloat16
EXP = mybir.ActivationFunctionType.Exp
ADD = mybir.AluOpType.add


def build_module(repeat: int = 1):
    nc = bacc.Bacc("TRN2", target_bir_lowering=False, debug=False,
                   num_devices=NCORES)

    xt_d = nc.dram_tensor("xt", [C, N], f32, kind="ExternalInput")
    xo_d = nc.dram_tensor("xo", [C, N], f32, kind="ExternalInput")
    thwT_d = nc.dram_tensor("thwT", [C, I], bf16, kind="ExternalInput")
    phwT_d = nc.dram_tensor("phwT", [C, I], bf16, kind="ExternalInput")
    gwT_d = nc.dram_tensor("gwT", [C, I], bf16, kind="ExternalInput")
    weffT_d = nc.dram_tensor("weffT", [I, C], bf16, kind="ExternalInput")
    tb_d = nc.dram_tensor("tb", [I, 1], f32, kind="ExternalInput")
    pb_d = nc.dram_tensor("pb", [I, 1], f32, kind="ExternalInput")
    beff_d = nc.dram_tensor("beff", [128, 2], f32, kind="ExternalInput")
    ident_d = nc.dram_tensor("ident", [128, 128], bf16, kind="ExternalInput")
    out_d = nc.dram_tensor("out", [C, N], f32, kind="ExternalOutput")

    # DRAM views with the c dim split as c = a*128 + p  (p = partition)
    xt_v = xt_d.ap().rearrange("(a p) n -> p a n", p=128)
    xo_v = xo_d.ap().rearrange("(a p) n -> p a n", p=128)
    out_v = out_d.ap().rearrange("(a p) n -> p a n", p=128)

    with tile.TileContext(nc) as tc:
        with (
            tc.tile_pool(name="const", bufs=1) as constp,
            tc.tile_pool(name="persist", bufs=1) as persist,
            tc.tile_pool(name="stage", bufs=3) as stagep,
            tc.tile_pool(name="big", bufs=2) as bigp,
            tc.tile_pool(name="ysmall", bufs=4) as ypool,
            tc.tile_pool(name="ytp", bufs=2) as ytpool,
            tc.tile_pool(name="outp", bufs=3) as outp,
            tc.tile_pool(name="pst", bufs=2, space="PSUM") as psum_st,
            tc.tile_pool(name="psm", bufs=2, space="PSUM") as psum_sm,
            tc.tile_pool(name="poc", bufs=2, space="PSUM") as psum_oc,
        ):
            # ---- weights / constants (loaded once) ----
            thwT = constp.tile([128, 2, I], bf16, tag="thwT")
            nc.sync.dma_start(out=thwT,
                              in_=thwT_d.ap().rearrange("(a p) i -> p a i", p=128))
            phwT = constp.tile([128, 2, I], bf16, tag="phwT")
            nc.sync.dma_start(out=phwT,
                              in_=phwT_d.ap().rearrange("(a p) i -> p a i", p=128))
            gwT = constp.tile([128, 2, I], bf16, tag="gwT")
            nc.sync.dma_start(out=gwT,
                              in_=gwT_d.ap().rearrange("(a p) i -> p a i", p=128))
            weffT = constp.tile([128, 2, 128], bf16, tag="weffT")
            nc.sync.dma_start(out=weffT,
                              in_=weffT_d.ap().rearrange("i (h c) -> i h c", h=2))
            tb = constp.tile([128, 1], f32, tag="tb")
            nc.sync.dma_start(out=tb, in_=tb_d.ap())
            pb = constp.tile([128, 1], f32, tag="pb")
            nc.sync.dma_start(out=pb, in_=pb_d.ap())
            beff = constp.tile([128, 2], f32, tag="beff")
            nc.sync.dma_start(out=beff, in_=beff_d.ap())
            ident = constp.tile([128, 128], bf16, tag="ident")
            nc.sync.dma_start(out=ident, in_=ident_d.ap())

            for _rep in range(repeat):
                xt = persist.tile([128, 2, N], f32, tag="xt")
                xtb = persist.tile([128, 2, N], bf16, tag="xtb")
                xob = persist.tile([128, 2, N], bf16, tag="xob")
                theta = persist.tile([128, N], bf16, tag="theta")
                phi = persist.tile([128, N], bf16, tag="phi")
                gTo = persist.tile([128, MT, 132], bf16, tag="gTo")
                nc.vector.memset(gTo[:, :, 128:129], 1.0)

                # ---- load x, cast to bf16, 1x1 convs ----
                for j in range(NCH):
                    for a in range(2):
                        nc.sync.dma_start(out=xt[:, a, ts(j, 512)],
                                          in_=xt_v[:, a, ts(j, 512)])
                    xos = stagep.tile([128, 2, 512], f32, tag="xos")
                    for a in range(2):
                        nc.sync.dma_start(out=xos[:, a, :],
                                          in_=xo_v[:, a, ts(j, 512)])
                    nc.vector.tensor_copy(xtb[:, :, ts(j, 512)],
                                          xt[:, :, ts(j, 512)])
                    nc.vector.tensor_copy(xob[:, :, ts(j, 512)], xos[:])

                for j in range(NCH):
                    # theta conv chunk
                    ps_t = psum_oc.tile([128, 512], f32, tag="oc")
                    for a in range(2):
                        nc.tensor.matmul(ps_t[:],
                                         lhsT=thwT[:, a, :],
                                         rhs=xtb[:, a, ts(j, 512)],
                                         start=(a == 0), stop=(a == 1))
                    nc.vector.tensor_scalar_add(theta[:, ts(j, 512)], ps_t[:], tb[:])
                    # phi conv chunk
                    ps_p = psum_oc.tile([128, 512], f32, tag="oc")
                    for a in range(2):
                        nc.tensor.matmul(ps_p[:],
                                         lhsT=phwT[:, a, :],
                                         rhs=xob[:, a, ts(j, 512)],
                                         start=(a == 0), stop=(a == 1))
                    nc.vector.tensor_scalar_add(phi[:, ts(j, 512)], ps_p[:], pb[:])
                    # gT conv for the 4 m-tiles inside this chunk
                    for t in range(4 * j, 4 * j + 4):
                        pg = psum_sm.tile([128, 132], f32, tag="sm")
                        for a in range(2):
                            nc.tensor.matmul(pg[:, 0:128],
                                             lhsT=xob[:, a, ts(t, 128)],
                                             rhs=gwT[:, a, :],
                                             start=(a == 0), stop=(a == 1))
                        nc.vector.tensor_copy(gTo[:, t, 0:128], pg[:, 0:128])

                # ---- attention + output, n-chunk at a time ----
                for j in range(NCH):
                    PT = bigp.tile([128, MT, 512], bf16, tag="big")
                    for t2 in range(MT // 2):
                        pss = psum_st.tile([128, 2, 512], f32, tag="st")
                        for q in range(2):
                            t = 2 * t2 + q
                            nc.tensor.matmul(pss[:, q, :],
                                             lhsT=phi[:, ts(t, 128)],
                                             rhs=theta[:, ts(j, 512)],
                                             start=True, stop=True)
                        nc.scalar.activation(PT[:, 2 * t2:2 * t2 + 2, :], pss[:],
                                             EXP)

                    yT = ytpool.tile([128, 512], bf16, tag="yT")
                    for s in range(4):
                        pv = psum_sm.tile([128, 132], f32, tag="sm")
                        for t in range(MT):
                            nc.tensor.matmul(pv[:, 0:129],
                                             lhsT=PT[:, t, ts(s, 128)],
                                             rhs=gTo[:, t, 0:129],
                                             start=(t == 0), stop=(t == MT - 1))
                        rcp = ypool.tile([128, 1], f32, tag="rcp")
                        nc.vector.reciprocal(rcp[:], pv[:, 128:129])
                        y = ypool.tile([128, 128], bf16, tag="y")
                        nc.vector.tensor_scalar_mul(y[:], pv[:, 0:128], rcp[:])
                        ytp = psum_sm.tile([128, 128], bf16, tag="sm")
                        nc.tensor.transpose(ytp[:], y[:], ident[:])
                        nc.vector.tensor_copy(yT[:, ts(s, 128)], ytp[:])

                    for h in range(2):
                        oc = psum_oc.tile([128, 512], f32, tag="oc")
                        nc.tensor.matmul(oc[:], lhsT=weffT[:, h, :], rhs=yT[:],
                                         start=True, stop=True)
                        ob = outp.tile([128, 512], f32, tag="ob")
                        nc.vector.scalar_tensor_tensor(
                            ob[:], oc[:], beff[:, h:h + 1],
                            xt[:, h, ts(j, 512)], op0=ADD, op1=ADD)
                        nc.sync.dma_start(out=out_v[:, h, ts(j, 512)], in_=ob[:])

    nc.compile()
    return nc


_CACHE: dict = {}


def _get_built(repeat: int = 1):
    if repeat not in _CACHE:
        _CACHE[repeat] = build_module(repeat)
    return _CACHE[repeat]


def prep_maps(inputs: dict) -> list[dict]:
    """Host-side precompute: fold BN + g/out biases, transpose weights."""
    f = lambda k: np.asarray(inputs[k], np.float32)
    x_this = f("x_this").reshape(B, C, N)
    x_other = f("x_other").reshape(B, C, N)
    theta_w, theta_b = f("theta_w"), f("theta_b")
    phi_w, phi_b = f("phi_w"), f("phi_b")
    g_w, g_b = f("g_w"), f("g_b")
    out_w, out_b = f("out_w"), f("out_b")
    gam, bet = f("bn_gamma"), f("bn_beta")
    mean, var = f("bn_mean"), f("bn_var")

    s = (gam / np.sqrt(var + BN_EPS)).astype(np.float32)  # [C]
    w_eff = (out_w * s[:, None]).astype(np.float32)  # [C, I]
    b_eff = (s * (out_w @ g_b + out_b - mean) + bet).astype(np.float32)  # [C]

    bf = ml_dtypes.bfloat16
    common = {
        "thwT": np.ascontiguousarray(theta_w.T).astype(bf),
        "phwT": np.ascontiguousarray(phi_w.T).astype(bf),
        "gwT": np.ascontiguousarray(g_w.T).astype(bf),
        "weffT": np.ascontiguousarray(w_eff.T).astype(bf),
        "tb": np.ascontiguousarray(theta_b[:, None]),
        "pb": np.ascontiguousarray(phi_b[:, None]),
        "beff": np.ascontiguousarray(b_eff.reshape(2, 128).T),
        "ident": np.eye(128, dtype=bf),
    }
    return [
        {"xt": np.ascontiguousarray(x_this[b]),
         "xo": np.ascontiguousarray(x_other[b]), **common}
        for b in range(B)
    ]


def run(inputs: dict, repeat: int = 1, time_it: bool = False):
    nc = _get_built(repeat)
    maps = prep_maps(inputs)
    t0 = time.time()
    res = run_bass_kernel_spmd(nc, maps, list(range(NCORES)))
    wall = time.time() - t0
    out = np.stack([np.asarray(res.results[b]["out"], np.float32)
                    for b in range(B)])
    out = out.reshape(B, C, HH, WW)
    if time_it:
        return out, wall
    return out


def kernel(**inputs) -> np.ndarray:
    return run(inputs)
